# revision 1
# baseline (speedup 1.0000x reference)
"""Trainium2 Bass kernel for nn_Attention_Layer_78855599554595.

GQA attention layer: QKV proj -> causal GQA attention (16 heads, 4 kv heads,
E=128) -> out proj -> exact GELU -> residual -> LayerNorm.  B=2, L=2048, D=2048.

Sharding: zero-communication interleaved sequence parallelism.
  - 8 cores = 2 batches x 4 cores/batch.
  - Core j of a batch owns query rows in g=64-row blocks strided by 4:
    global blocks {j, j+4, ..., j+28} (512 rows).  This makes the causal
    work identical across cores (SPMD: one program, per-core data): for
    key block kb (256 keys), exactly rows [64*kb, 512) (clamped at 256)
    of the core's permuted Q buffer attend to it; boundary blocks get a
    host-supplied additive mask.
  - Each core computes K/V for its full batch (redundant 4x, but cheaper
    than any on-chip collective: a 16MB ReduceScatter is ~500us here),
    Q/attention/out-proj/GELU/residual/LN only for its 512 rows.
  - Scores are computed transposed (S^T: keys on partitions, queries on
    the free axis) so softmax needs no transposes: the key-sum is a
    ones-vector matmul on the PE, and no max-subtraction is needed
    (scores are O(+-10) for this distribution; exp is fp32-safe).
  - All matmuls run in float32r (TF32-class precision at full PE rate).

Host-side (free): x transposes/gathers, mask construction, gamma/beta/bo
broadcast, output reassembly.
"""

import sys

sys.path.insert(0, "/opt/trn_rl_repo")

import numpy as np

from contextlib import ExitStack
from dataclasses import dataclass, field

from concourse import bacc, mybir, tile
from concourse.masks import make_identity

F32 = mybir.dt.float32
NEG = -1.0e9
AF = mybir.ActivationFunctionType


@dataclass(frozen=True)
class Cfg:
    L: int = 2048          # sequence length (per batch)
    D: int = 2048          # model dim
    H: int = 16            # query heads
    KV: int = 4            # kv heads
    E: int = 128           # head dim (= partition width)
    mm_dt: object = field(default=mybir.dt.float32r)
    act: object = field(default=None)  # None -> exact GELU
    trivial_affine: bool = False  # gamma==1, beta==0, bo==0: skip those ops

    @property
    def g(self):           # q block granularity (32 blocks across L)
        return self.L // 32

    @property
    def KB(self):          # key block size = 4*g
        return self.L // 8

    @property
    def KSS(self):         # key subtile (partition) size
        return min(self.KB, 128)

    @property
    def ST(self):          # key subtiles per key block
        return max(1, self.KB // 128)

    @property
    def QR(self):          # query rows per core
        return self.L // 4

    @property
    def KT(self):          # contraction tiles over D
        return self.D // 128

    @property
    def RT(self):          # 128-row tiles of the core's q rows
        return self.QR // 128

    @property
    def RC(self):          # row-chunk size for K/V projection
        return min(self.L, 256)

    @property
    def OC(self):          # out-proj / LN column chunk
        return min(self.D, 512)


def build_program(cfg: Cfg):
    """Build the single-core SPMD Bass program. Returns finalized nc."""
    L, D, H, KV, E = cfg.L, cfg.D, cfg.H, cfg.KV, cfg.E
    g, KB, KSS, ST, QR, KT, RT = (cfg.g, cfg.KB, cfg.KSS, cfg.ST, cfg.QR,
                                  cfg.KT, cfg.RT)
    RC, OC = cfg.RC, cfg.OC
    NRC = L // RC
    NOC = D // OC
    R = cfg.mm_dt
    act_fn = cfg.act if cfg.act is not None else AF.Gelu
    inv_sqrt_e = 1.0 / float(np.sqrt(E))

    nc = bacc.Bacc(None, target_bir_lowering=False)

    # ---- DRAM I/O (per-core data; same names on every core) ----
    xt = nc.dram_tensor("xt", [D, L], F32, kind="ExternalInput")      # x[b].T
    xtq = nc.dram_tensor("xtq", [D, QR], F32, kind="ExternalInput")   # cols at q rows
    xq = nc.dram_tensor("xq", [QR, D], F32, kind="ExternalInput")     # rows at q rows
    wq = nc.dram_tensor("wq", [D, H * E], F32, kind="ExternalInput")
    wk = nc.dram_tensor("wk", [D, KV * E], F32, kind="ExternalInput")
    wv = nc.dram_tensor("wv", [D, KV * E], F32, kind="ExternalInput")
    wo = nc.dram_tensor("wo", [H * E, D], F32, kind="ExternalInput")
    bqT = nc.dram_tensor("bqT", [E, H], F32, kind="ExternalInput")
    bkb = nc.dram_tensor("bkb", [128, KV * E], F32, kind="ExternalInput")
    bvb = nc.dram_tensor("bvb", [128, KV * E], F32, kind="ExternalInput")
    bo2 = nc.dram_tensor("bo2", [2, D], F32, kind="ExternalInput")  # bo row + zero row
    gmb = nc.dram_tensor("gmb", [128, D], F32, kind="ExternalInput")  # gamma bcast
    btb = nc.dram_tensor("btb", [128, D], F32, kind="ExternalInput")  # beta bcast
    # masks in S^T layout (keys, q), additive, raw (pre-softmax-scale)
    maskd = nc.dram_tensor("maskd", [KB, g], F32, kind="ExternalInput")
    maskp = nc.dram_tensor("maskp", [4, KB, QR // 2], F32, kind="ExternalInput")
    out = nc.dram_tensor("out", [QR, D], F32, kind="ExternalOutput")

    with tile.TileContext(nc) as tc, ExitStack() as top:
        # ---- persistent pools (stack order matters for SBUF reuse) ----
        const = top.enter_context(tc.tile_pool(name="const", bufs=1))
        qt_stack = top.enter_context(ExitStack())
        qt_pool = qt_stack.enter_context(tc.tile_pool(name="qtp", bufs=1))
        kvq_stack = ExitStack()
        kvq_pool = kvq_stack.enter_context(tc.tile_pool(name="kvq", bufs=1))

        # constants
        ones_r = const.tile([128, 2], R)
        ones_f = const.tile([128, 2], F32)
        nc.gpsimd.memset(ones_f[:], 1.0)
        nc.vector.tensor_copy(ones_r[:], ones_f[:])
        ones_row = const.tile([128, 128], R)
        ones_rowf = const.tile([128, 128], F32)
        nc.gpsimd.memset(ones_rowf[:], 0.0)
        nc.gpsimd.memset(ones_rowf[:1, :], 1.0)
        nc.gpsimd.memset(ones_rowf[64:65, :], 1.0)
        nc.vector.tensor_copy(ones_row[:], ones_rowf[:])
        ident = const.tile([128, 128], F32)
        make_identity(nc, ident)

        bq_t = const.tile([E, H], F32)
        bkb_t = const.tile([128, KV * E], F32)
        bvb_t = const.tile([128, KV * E], F32)
        nc.sync.dma_start(out=bq_t[:], in_=bqT[:])
        nc.sync.dma_start(out=bkb_t[:], in_=bkb[:])
        nc.sync.dma_start(out=bvb_t[:], in_=bvb[:])

        maskd_t = const.tile([KSS, ST, g], F32)
        maskp_t = const.tile([KSS, 4, ST, QR // 2], F32)
        nc.sync.dma_start(out=maskd_t[:],
                          in_=maskd.rearrange("(s p) q -> p s q", p=KSS))
        nc.sync.dma_start(out=maskp_t[:],
                          in_=maskp.rearrange("n (s p) q -> p n s q", p=KSS))

        # persistent activations: K^T, V (natural) per kv head
        kT = [kvq_pool.tile([E, L], R, tag=f"kT{kv}", name=f"kT{kv}") for kv in range(KV)]
        vN = [kvq_pool.tile([KSS, L // KSS, E], R, tag=f"vN{kv}", name=f"vN{kv}")
              for kv in range(KV)]

        # ================= Phase 1: K/V projections (full batch rows) ======
        # x^T tiles stationary; K/V come out natural (rows x kvE); K is then
        # PE-transposed into kT.
        with ExitStack() as ph:
            wkv_pool = ph.enter_context(tc.tile_pool(name="wkv", bufs=1))
            stage = ph.enter_context(tc.tile_pool(name="stage1", bufs=3))
            ps1 = ph.enter_context(tc.tile_pool(name="ps1", bufs=2, space="PSUM"))
            pst = ph.enter_context(tc.tile_pool(name="pst", bufs=2, space="PSUM"))
            ev1 = ph.enter_context(tc.tile_pool(name="ev1", bufs=1))

            wk_r = wkv_pool.tile([128, KT, KV * E], R)
            wv_r = wkv_pool.tile([128, KT, KV * E], R)
            for kt in range(KT):
                wst = stage.tile([128, KV * E], F32, tag="wst")
                nc.sync.dma_start(out=wst[:], in_=wk[kt * 128:(kt + 1) * 128, :])
                nc.vector.tensor_copy(wk_r[:, kt, :], wst[:])
            for kt in range(KT):
                wst2 = stage.tile([128, KV * E], F32, tag="wst")
                nc.sync.dma_start(out=wst2[:], in_=wv[kt * 128:(kt + 1) * 128, :])
                nc.vector.tensor_copy(wv_r[:, kt, :], wst2[:])

            KH = max(KT // 2, 1)
            for rt in range(L // 128):
                xsrs = []
                for half in range(KT // KH):
                    xsl = stage.tile([128, KH, 128], F32, tag="xsl")
                    nc.sync.dma_start(
                        out=xsl[:],
                        in_=xt[half * KH * 128:(half + 1) * KH * 128,
                               rt * 128:(rt + 1) * 128]
                        .rearrange("(kt p) r -> p kt r", p=128))
                    xsr = stage.tile([128, KH, 128], R, tag="xsr")
                    nc.vector.tensor_copy(xsr[:], xsl[:])
                    xsrs.append(xsr)
                pK = ps1.tile([128, KV * E], F32, tag="pK")
                pV = ps1.tile([128, KV * E], F32, tag="pV")
                for kt in range(KT):
                    nc.tensor.matmul(pK[:], xsrs[kt // KH][:, kt % KH, :],
                                     wk_r[:, kt, :],
                                     start=(kt == 0), stop=(kt == KT - 1))
                for kt in range(KT):
                    nc.tensor.matmul(pV[:], xsrs[kt // KH][:, kt % KH, :],
                                     wv_r[:, kt, :],
                                     start=(kt == 0), stop=(kt == KT - 1))
                # V natural: evict (+bias) straight into vN, rounding to fp32r
                for kv in range(KV):
                    for piece in range(128 // KSS):
                        nc.vector.tensor_add(
                            vN[kv][:, rt * (128 // KSS) + piece, :],
                            pV[piece * KSS:(piece + 1) * KSS,
                               kv * E:(kv + 1) * E],
                            bvb_t[piece * KSS:(piece + 1) * KSS,
                                  kv * E:(kv + 1) * E])
                # K natural: bias-add to SBUF, then PE-transpose into kT
                knat = ev1.tile([128, KV * E], F32, tag="knat")
                nc.vector.tensor_add(knat[:], pK[:], bkb_t[:])
                for kv in range(KV):
                    ptr = pst.tile([128, 128], F32, tag="ptr")
                    nc.tensor.transpose(
                        ptr[:], knat[:, kv * E:(kv + 1) * E], ident[:])
                    nc.scalar.activation(
                        kT[kv][:, rt * 128:(rt + 1) * 128], ptr[:], AF.Copy)

        # ================= Phase 2: Q^T projection (core's rows) ===========
        qT = [qt_pool.tile([E, QR], R, tag=f"qT{h}", name=f"qT{h}") for h in range(H)]
        with ExitStack() as ph:
            stage = ph.enter_context(tc.tile_pool(name="stage2", bufs=4))
            xtq_pool = ph.enter_context(tc.tile_pool(name="xtqp", bufs=1))
            ps2 = ph.enter_context(tc.tile_pool(name="ps2", bufs=1, space="PSUM"))

            xtq_r = xtq_pool.tile([128, KT, QR], R)
            for kt in range(KT):
                xst = stage.tile([128, QR], F32, tag="xst2")
                nc.gpsimd.dma_start(out=xst[:], in_=xtq[kt * 128:(kt + 1) * 128, :])
                nc.vector.tensor_copy(xtq_r[:, kt, :], xst[:])
            HB = 8 if H % 8 == 0 else 4
            for hb in range(H // HB):
                pqs = [ps2.tile([E, QR], F32, tag=f"pq{hh}", name=f"pq{hh}")
                       for hh in range(HB)]
                for kt in range(KT):
                    wqs = stage.tile([128, HB * E], F32, tag="wqs")
                    nc.gpsimd.dma_start(
                        out=wqs[:],
                        in_=wq[kt * 128:(kt + 1) * 128,
                               hb * HB * E:(hb + 1) * HB * E])
                    wqr = stage.tile([128, HB * E], R, tag="wqr")
                    nc.vector.tensor_copy(wqr[:], wqs[:])
                    for hh in range(HB):
                        nc.tensor.matmul(
                            pqs[hh][:], wqr[:, hh * E:(hh + 1) * E],
                            xtq_r[:, kt, :],
                            start=(kt == 0), stop=(kt == KT - 1))
                for hh in range(HB):
                    h = hb * HB + hh
                    nc.scalar.activation(
                        qT[h][:], pqs[hh][:], AF.Identity, bias=bq_t[:, h:h + 1])

        # ================= Phase 3: attention ==============================
        ctxT = [None] * H
        with ExitStack() as ph:
            ps_ctx = ph.enter_context(
                tc.tile_pool(name="psctx", bufs=2, space="PSUM"))
            ps_l = ph.enter_context(tc.tile_pool(name="psl", bufs=2, space="PSUM"))
            ps_s = ph.enter_context(tc.tile_pool(name="pss", bufs=3, space="PSUM"))
            ps_rb = ph.enter_context(tc.tile_pool(name="psrb", bufs=1, space="PSUM"))
            exp_pool = ph.enter_context(tc.tile_pool(name="expp", bufs=6))
            lso_pool = ph.enter_context(tc.tile_pool(name="lso", bufs=2))

            for h in range(H):
                kv = h % KV
                pctx = ps_ctx.tile([E, QR], F32, tag="pctx")
                pl = ps_l.tile([2, QR], F32, tag="pl")
                first = True
                for kb in range(8):
                    q0 = min(g * kb, QR // 2)
                    qc = QR - q0
                    for st in range(ST):
                        k0 = kb * KB + st * KSS
                        pS = ps_s.tile([KSS, QR], F32, tag="pS")
                        nc.tensor.matmul(pS[:, :qc], kT[kv][:, k0:k0 + KSS],
                                         qT[h][:, q0:], start=True, stop=True)
                        if kb < 4:
                            nc.vector.tensor_add(
                                pS[:, :g], pS[:, :g], maskd_t[:, st, :])
                        else:
                            w = g * (kb - 3)
                            nc.vector.tensor_add(
                                pS[:, :w], pS[:, :w],
                                maskp_t[:, kb - 4, st, :w])
                        eS = exp_pool.tile([KSS, QR], R, tag="eS")
                        nc.scalar.activation(eS[:, :qc], pS[:, :qc], AF.Exp,
                                             scale=inv_sqrt_e)
                        last = (kb == 7 and st == ST - 1)
                        nc.tensor.matmul(pl[:, q0:], ones_r[:KSS, :],
                                         eS[:, :qc], start=first, stop=last,
                                         skip_group_check=True)
                        nc.tensor.matmul(pctx[:, q0:], vN[kv][:, k0 // KSS, :],
                                         eS[:, :qc], start=first, stop=last,
                                         skip_group_check=True)
                        first = False
                l2f = lso_pool.tile([2, QR], F32, tag="ls")
                rl2 = lso_pool.tile([2, QR], R, tag="rl")
                nc.gpsimd.memset(l2f[:], 0.0)
                nc.vector.reciprocal_approx_fast(l2f[:1, :], pl[:1, :])
                nc.vector.tensor_copy(rl2[:], l2f[:])
                prb = ps_rb.tile([E, QR], F32, tag="prb")
                nc.tensor.matmul(prb[:], ones_row[:2, :], rl2[:],
                                 start=True, stop=True)
                rb_s = lso_pool.tile([E, QR], F32, tag="rbs")
                nc.scalar.activation(rb_s[:], prb[:], AF.Copy)
                cT = qt_pool.tile([E, QR], R, tag=f"qT{h}", name=f"cT{h}")
                nc.vector.tensor_mul(cT[:], pctx[:], rb_s[:])
                ctxT[h] = cT

        kvq_stack.close()

        # ================= Phase 4: out-proj + GELU + residual =============
        r_stack = top.enter_context(ExitStack())
        rfull_pool = r_stack.enter_context(tc.tile_pool(name="rfull", bufs=1))
        stat4 = r_stack.enter_context(tc.tile_pool(name="stat4", bufs=1))
        r_full = [rfull_pool.tile([128, D], F32, tag=f"rf{rt}", name=f"rf{rt}")
                  for rt in range(RT)]
        with ExitStack() as ph:
            wo_pool = ph.enter_context(tc.tile_pool(name="wop", bufs=2))
            wstg = ph.enter_context(tc.tile_pool(name="wstg", bufs=2))
            ps_y = ph.enter_context(tc.tile_pool(name="psy", bufs=1, space="PSUM"))
            ep_pool = ph.enter_context(tc.tile_pool(name="epp", bufs=3))
            cst = ph.enter_context(tc.tile_pool(name="cst4", bufs=1))

            bo2f = cst.tile([2, D], F32)
            nc.sync.dma_start(out=bo2f[:], in_=bo2[:])
            bo2r = cst.tile([2, D], R)
            nc.vector.tensor_copy(bo2r[:], bo2f[:])

            HW4 = 4  # h-chunk per wo load piece
            part_sums, part_sqs = [], []
            for oc in range(NOC):
                # load wo[:, oc] transposed-tiled: (128, H, OC) in pieces
                woc = wo_pool.tile([128, H, OC], R, tag="woc")
                for pc in range(H // HW4):
                    wos = wstg.tile([128, HW4, OC], F32, tag="wos")
                    nc.sync.dma_start(
                        out=wos[:],
                        in_=wo[pc * HW4 * E:(pc + 1) * HW4 * E,
                               oc * OC:(oc + 1) * OC]
                        .rearrange("(h p) c -> p h c", p=128))
                    nc.vector.tensor_copy(
                        woc[:, pc * HW4:(pc + 1) * HW4, :], wos[:])
                pys = [ps_y.tile([128, OC], F32, tag=f"py{rt}", name=f"py{rt}")
                       for rt in range(RT)]
                for h in range(H):
                    for rt in range(RT):
                        nc.tensor.matmul(
                            pys[rt][:], ctxT[h][:, rt * 128:(rt + 1) * 128],
                            woc[:, h, :], start=(h == 0),
                            stop=(cfg.trivial_affine and h == H - 1))
                if not cfg.trivial_affine:
                    for rt in range(RT):
                        nc.tensor.matmul(
                            pys[rt][:], ones_row[:2, :128],
                            bo2r[:, oc * OC:(oc + 1) * OC],
                            start=False, stop=True)
                for rt in range(RT):
                    t2 = ep_pool.tile([128, OC], F32, tag="t2")
                    nc.scalar.activation(t2[:], pys[rt][:], act_fn)
                    xqt = ep_pool.tile([128, OC], F32, tag="xqt")
                    nc.gpsimd.dma_start(
                        out=xqt[:],
                        in_=xq[rt * 128:(rt + 1) * 128, oc * OC:(oc + 1) * OC])
                    rchunk = r_full[rt][:, oc * OC:(oc + 1) * OC]
                    nc.vector.tensor_add(rchunk, t2[:], xqt[:])
                    psum_t = stat4.tile([128, 1], F32, tag=f"psum{oc}_{rt}",
                                        name=f"psum{oc}_{rt}")
                    nc.vector.reduce_sum(psum_t[:], rchunk,
                                         axis=mybir.AxisListType.X)
                    psq_t = stat4.tile([128, 1], F32, tag=f"psq{oc}_{rt}",
                                       name=f"psq{oc}_{rt}")
                    jnk = ep_pool.tile([128, OC], F32, tag="jnk")
                    nc.scalar.activation(jnk[:], rchunk, AF.Square,
                                         accum_out=psq_t[:])
                    if oc == 0:
                        part_sums.append([psum_t])
                        part_sqs.append([psq_t])
                    else:
                        part_sums[rt].append(psum_t)
                        part_sqs[rt].append(psq_t)

        # ================= Phase 5: LayerNorm ==============================
        with ExitStack() as ph:
            ln_pool = ph.enter_context(tc.tile_pool(name="lnp", bufs=2))
            st_pool = ph.enter_context(tc.tile_pool(name="stp", bufs=2))
            gb_pool = ph.enter_context(tc.tile_pool(name="gbp", bufs=2))
            cst = ph.enter_context(tc.tile_pool(name="cst5", bufs=1))
            eps_t = cst.tile([128, 1], F32)
            nc.gpsimd.memset(eps_t[:], 1e-5)

            inv_d = 1.0 / D
            rstds, nmrs = [], []
            for rt in range(RT):
                ssum = st_pool.tile([128, 1], F32, tag="ssum")
                ssq = st_pool.tile([128, 1], F32, tag="ssq")
                if NOC > 1:
                    nc.vector.tensor_add(ssum[:], part_sums[rt][0][:],
                                         part_sums[rt][1][:])
                    nc.vector.tensor_add(ssq[:], part_sqs[rt][0][:],
                                         part_sqs[rt][1][:])
                    for c in range(2, NOC):
                        nc.vector.tensor_add(ssum[:], ssum[:],
                                             part_sums[rt][c][:])
                        nc.vector.tensor_add(ssq[:], ssq[:], part_sqs[rt][c][:])
                else:
                    nc.vector.tensor_copy(ssum[:], part_sums[rt][0][:])
                    nc.vector.tensor_copy(ssq[:], part_sqs[rt][0][:])
                mu = st_pool.tile([128, 1], F32, tag="mu")
                nc.vector.tensor_scalar_mul(mu[:], ssum[:], inv_d)
                ex2 = st_pool.tile([128, 1], F32, tag="ex2")
                nc.vector.tensor_scalar_mul(ex2[:], ssq[:], inv_d)
                mu2 = st_pool.tile([128, 1], F32, tag="mu2")
                nc.vector.tensor_mul(mu2[:], mu[:], mu[:])
                var = st_pool.tile([128, 1], F32, tag="var")
                nc.vector.tensor_sub(var[:], ex2[:], mu2[:])
                std = st_pool.tile([128, 1], F32, tag="std")
                nc.scalar.activation(std[:], var[:], AF.Sqrt, bias=eps_t[:])
                rstd = st_pool.tile([128, 1], F32, tag=f"rstd{rt}",
                                    name=f"rstd{rt}")
                nc.vector.reciprocal(rstd[:], std[:])
                nmr = st_pool.tile([128, 1], F32, tag=f"nmr{rt}",
                                   name=f"nmr{rt}")
                nc.vector.tensor_mul(nmr[:], mu[:], rstd[:])
                nc.vector.tensor_scalar_mul(nmr[:], nmr[:], -1.0)
                rstds.append(rstd)
                nmrs.append(nmr)
            for c in range(NOC):
                sl = slice(c * OC, (c + 1) * OC)
                if not cfg.trivial_affine:
                    gm_c = gb_pool.tile([128, OC], F32, tag="gmc")
                    bt_c = gb_pool.tile([128, OC], F32, tag="btc")
                    nc.sync.dma_start(out=gm_c[:], in_=gmb[:, sl])
                    nc.sync.dma_start(out=bt_c[:], in_=btb[:, sl])
                for rt in range(RT):
                    par = (c + rt) % 2
                    t = ln_pool.tile([128, OC], F32, tag="lt")
                    if par:
                        nc.scalar.activation(
                            t[:], r_full[rt][:, sl], AF.Identity,
                            scale=rstds[rt][:], bias=nmrs[rt][:])
                    else:
                        nc.vector.tensor_scalar(
                            t[:], r_full[rt][:, sl], rstds[rt][:], nmrs[rt][:],
                            op0=mybir.AluOpType.mult, op1=mybir.AluOpType.add)
                    if cfg.trivial_affine:
                        yf = t
                    else:
                        t2 = ln_pool.tile([128, OC], F32, tag="lt2")
                        nc.vector.tensor_mul(t2[:], t[:], gm_c[:])
                        yf = ln_pool.tile([128, OC], F32, tag="yf")
                        nc.vector.tensor_add(yf[:], t2[:], bt_c[:])
                    nc.sync.dma_start(out=out[rt * 128:(rt + 1) * 128, sl],
                                      in_=yf[:])

    nc.finalize()
    return nc


# ---------------------------------------------------------------------------
# host-side mask construction + sharding
# ---------------------------------------------------------------------------

def build_masks(cfg: Cfg, j: int):
    g, KB, QR = cfg.g, cfg.KB, cfg.QR
    c = np.arange(KB)[:, None]
    r = np.arange(g)[None, :]
    maskd = np.where(c <= j * g + r, 0.0, NEG).astype(np.float32)
    maskp = np.zeros((4, KB, QR // 2), np.float32)
    m = np.arange(QR // 2)
    i_of_m = 4 + m // g
    r_of_m = m % g
    for kbi, kb in enumerate(range(4, 8)):
        block = np.zeros((KB, QR // 2), np.float32)
        block[:, i_of_m < kb] = NEG
        dcols = np.where(i_of_m == kb)[0]
        block[:, dcols] = np.where(c <= j * g + r_of_m[dcols][None, :], 0.0, NEG)
        maskp[kbi] = block
    return maskd, maskp


def q_rows(cfg: Cfg, j: int):
    g = cfg.g
    return np.concatenate(
        [np.arange((j + 4 * i) * g, (j + 4 * i + 1) * g) for i in range(8)])


def make_in_map(cfg: Cfg, shared, x, b, j):
    rows = q_rows(cfg, j)
    xb = np.asarray(x, np.float32)[b]
    xbT = np.ascontiguousarray(xb.T)
    maskd, maskp = build_masks(cfg, j)
    return dict(
        shared,
        xt=xbT,
        xtq=np.ascontiguousarray(xbT[:, rows]),
        xq=np.ascontiguousarray(xb[rows]),
        maskd=maskd,
        maskp=maskp,
    )


def make_shared(cfg: Cfg, Wq, bq, Wk, bk, Wv, bv, Wo, bo, gamma, beta):
    H, KV, E, D = cfg.H, cfg.KV, cfg.E, cfg.D
    return {
        "wq": np.ascontiguousarray(Wq, dtype=np.float32),
        "wk": np.ascontiguousarray(Wk, dtype=np.float32),
        "wv": np.ascontiguousarray(Wv, dtype=np.float32),
        "wo": np.ascontiguousarray(Wo, dtype=np.float32),
        "bqT": np.ascontiguousarray(
            np.asarray(bq, np.float32).reshape(H, E).T),
        "bkb": np.ascontiguousarray(
            np.broadcast_to(np.asarray(bk, np.float32), (128, KV * E))),
        "bvb": np.ascontiguousarray(
            np.broadcast_to(np.asarray(bv, np.float32), (128, KV * E))),
        "bo2": np.ascontiguousarray(
            np.stack([np.asarray(bo, np.float32),
                      np.zeros(D, np.float32)])),
        "gmb": np.ascontiguousarray(
            np.broadcast_to(np.asarray(gamma, np.float32), (128, D))),
        "btb": np.ascontiguousarray(
            np.broadcast_to(np.asarray(beta, np.float32), (128, D))),
    }


def assemble(cfg: Cfg, results, B):
    out = np.empty((B, cfg.L, cfg.D), np.float32)
    for core in range(4 * B):
        b, j = divmod(core, 4)
        out[b, q_rows(cfg, j)] = results[core]["out"]
    return out


_NC_CACHE = {}


def kernel(x, Wq, bq, Wk, bk, Wv, bv, Wo, bo, gamma, beta):
    from concourse.bass_utils import run_bass_kernel_spmd

    trivial = bool(
        np.all(np.asarray(gamma) == 1.0) and np.all(np.asarray(beta) == 0.0)
        and np.all(np.asarray(bo) == 0.0))
    cfg = Cfg(trivial_affine=trivial)
    if cfg not in _NC_CACHE:
        _NC_CACHE[cfg] = build_program(cfg)
    nc = _NC_CACHE[cfg]
    shared = make_shared(cfg, Wq, bq, Wk, bk, Wv, bv, Wo, bo, gamma, beta)
    in_maps = [make_in_map(cfg, shared, x, *divmod(core, 4))
               for core in range(8)]
    res = run_bass_kernel_spmd(nc, in_maps, list(range(8)))
    return assemble(cfg, res.results, 2)

